# revision 17
# baseline (speedup 1.0000x reference)
"""ConvLSTM2D cell on 8 Trainium2 NeuronCores.

Data-parallel over batch: B=16 -> 2 images per core. Per core, the 3x3
conv over concat([x, h]) channels is computed as 14 PSUM-accumulated
matmul slots per (gate, 512-pixel chunk): 9 h-taps at K=128 plus the 9
x-taps packed into 4 K=128 matmuls + 1 K=64 matmul by baking shifted
copies of x into partitions 64-127 on the host (a fixed (dh,dw) shift
between the two channel halves lets one matmul contract two conv taps).
The padded image lives in SBUF so every tap is a pure access-pattern
shift. Matmuls default to fp16 (2 elem/cycle PE streaming, ~1e-3
scale-relative error; float32r available for ~5e-4 at 1.6x the time).
ScalarE applies bias+sigmoid/tanh straight out of PSUM; VectorE does
the LSTM elementwise math.
"""

import sys

if "/opt/trn_rl_repo" not in sys.path:
    sys.path.insert(0, "/opt/trn_rl_repo")

import numpy as np

import concourse.bass as bass
import concourse.tile as tile
from concourse import bacc, mybir
from concourse.bass_utils import run_bass_kernel_spmd

N_CORES = 8
B, C_IN, C_HID, H, W = 16, 64, 128, 64, 64
B_LOC = B // N_CORES  # 2 images per core
HP = H + 2  # padded
WP = W + 2
ROWS_PER_CHUNK = 8  # 8 rows x 64 cols = 512 pixels per matmul chunk
NCH = H // ROWS_PER_CHUNK  # chunks per image
TAPS = [(kh, kw) for kh in range(3) for kw in range(3)]

_cache = {}


def _build(dt_mm=mybir.dt.float32r, trace=False, unroll=1):
    key = (dt_mm, trace, unroll)
    if key in _cache:
        return _cache[key]
    f32 = mybir.dt.float32
    nc = bacc.Bacc("TRN2", target_bir_lowering=False, debug=False, num_devices=N_CORES)

    # x is host-duplicated: channels 0-63 = padded x, channels 64-127 = the
    # same image pre-shifted one column left. A single K=128 matmul against
    # stacked (kh,0)+(kh,1) weights then covers two conv taps at once.
    # x2: channels 0-63 = padded x, 64-127 = pre-shifted one ROW up, pairing
    # taps (0,2)+(1,2) the same way; (2,2) stays a lone K=64 matmul.
    x_ap = nc.dram_tensor("x", [B_LOC, 2 * C_IN, HP, WP], dt_mm, kind="ExternalInput").ap()
    x2_ap = nc.dram_tensor("x2", [B_LOC, 2 * C_IN, HP, WP], dt_mm, kind="ExternalInput").ap()
    h_ap = nc.dram_tensor("h", [B_LOC, C_HID, HP, WP], dt_mm, kind="ExternalInput").ap()
    c_ap = nc.dram_tensor("c", [B_LOC, C_HID, H * W], f32, kind="ExternalInput").ap()
    # wxp[kh]: stacked [wx(kh,0); wx(kh,1)] (K=128); wxr: [wx(0,2); wx(1,2)]
    # (K=128, row pair); wx3: wx(2,2) (K=64)
    wxp_ap = nc.dram_tensor("wxp", [3, 2 * C_IN, 4 * C_HID], dt_mm, kind="ExternalInput").ap()
    wxr_ap = nc.dram_tensor("wxr", [2 * C_IN, 4 * C_HID], dt_mm, kind="ExternalInput").ap()
    wx3_ap = nc.dram_tensor("wx3", [C_IN, 4 * C_HID], dt_mm, kind="ExternalInput").ap()
    wh_ap = nc.dram_tensor("wh", [9, C_HID, 4 * C_HID], dt_mm, kind="ExternalInput").ap()
    bias_ap = nc.dram_tensor("biasT", [C_HID, 4], f32, kind="ExternalInput").ap()
    hn_ap = nc.dram_tensor("hn", [B_LOC, C_HID, H * W], f32, kind="ExternalOutput").ap()
    cn_ap = nc.dram_tensor("cn", [B_LOC, C_HID, H * W], f32, kind="ExternalOutput").ap()

    with tile.TileContext(nc) as tc:
        with (
            tc.tile_pool(name="weights", bufs=1) as wpool,
            tc.tile_pool(name="imgs", bufs=2) as ipool,
            tc.tile_pool(name="cstate", bufs=3) as cpool,
            tc.tile_pool(name="psum", bufs=8, space="PSUM") as ppool,
            tc.tile_pool(name="acts", bufs=2) as apool,
            tc.tile_pool(name="outs", bufs=3) as opool,
        ):
            wh_t = wpool.tile([C_HID, 9, 4 * C_HID], dt_mm, tag="wh")
            wxp_t = wpool.tile([2 * C_IN, 3, 4 * C_HID], dt_mm, tag="wxp")
            wxr_t = wpool.tile([2 * C_IN, 4 * C_HID], dt_mm, tag="wxr")
            wx3_t = wpool.tile([C_IN, 4 * C_HID], dt_mm, tag="wx3")
            bias_t = wpool.tile([C_HID, 4], f32, tag="bias")
            nc.sync.dma_start(wh_t[:], wh_ap.rearrange("t k m -> k t m"))
            nc.sync.dma_start(wxp_t[:], wxp_ap.rearrange("t k m -> k t m"))
            nc.sync.dma_start(wxr_t[:], wxr_ap[:])
            nc.sync.dma_start(wx3_t[:], wx3_ap[:])
            nc.sync.dma_start(bias_t[:], bias_ap[:])

            hp = []
            xp = []
            xq = []
            for b in range(B_LOC):
                hp_b = ipool.tile([C_HID, HP, WP], dt_mm, tag="hp")
                xp_b = ipool.tile([2 * C_IN, HP, WP], dt_mm, tag="xp")
                xq_b = ipool.tile([2 * C_IN, HP, WP], dt_mm, tag="xq")
                nc.sync.dma_start(hp_b[:], h_ap[b])
                nc.sync.dma_start(xp_b[:], x_ap[b])
                nc.sync.dma_start(xq_b[:], x2_ap[b])
                hp.append(hp_b)
                xp.append(xp_b)
                xq.append(xq_b)

            for _rep in range(unroll):
                for b in range(B_LOC):
                    for ch in range(NCH):
                        h0 = ch * ROWS_PER_CHUNK
                        c_sl = cpool.tile([C_HID, 512], f32, tag="c", name=f"c_{_rep}_{b}_{ch}")
                        nc.sync.dma_start(c_sl[:], c_ap[b][:, h0 * W : (h0 + ROWS_PER_CHUNK) * W])
                        gate_sb = []
                        for g in range(4):
                            acc = ppool.tile([C_HID, 512], f32, tag="acc")
                            n_mm = 14
                            mm = 0
                            for kh, kw in TAPS:
                                nc.tensor.matmul(
                                    acc[:],
                                    wh_t[:, kh * 3 + kw, g * C_HID : (g + 1) * C_HID],
                                    hp[b][:, h0 + kh : h0 + kh + ROWS_PER_CHUNK, kw : kw + W],
                                    start=(mm == 0),
                                    stop=(mm == n_mm - 1),
                                )
                                mm += 1
                            for kh in range(3):
                                # taps (kh,0)+(kh,1) in one K=128 matmul
                                nc.tensor.matmul(
                                    acc[:],
                                    wxp_t[:, kh, g * C_HID : (g + 1) * C_HID],
                                    xp[b][:, h0 + kh : h0 + kh + ROWS_PER_CHUNK, 0:W],
                                    start=(mm == 0),
                                    stop=(mm == n_mm - 1),
                                )
                                mm += 1
                            # taps (0,2)+(1,2) in one K=128 matmul (row pair)
                            nc.tensor.matmul(
                                acc[:],
                                wxr_t[:, g * C_HID : (g + 1) * C_HID],
                                xq[b][:, h0 : h0 + ROWS_PER_CHUNK, 2 : 2 + W],
                                start=(mm == 0),
                                stop=(mm == n_mm - 1),
                            )
                            mm += 1
                            # tap (2,2), lone K=64 on the un-shifted channels
                            nc.tensor.matmul(
                                acc[:],
                                wx3_t[:, g * C_HID : (g + 1) * C_HID],
                                xp[b][0:C_IN, h0 + 2 : h0 + 2 + ROWS_PER_CHUNK, 2 : 2 + W],
                                start=(mm == 0),
                                stop=(mm == n_mm - 1),
                            )
                            mm += 1
                            act_t = apool.tile([C_HID, 512], f32, tag=f"gate{g}")
                            func = (
                                mybir.ActivationFunctionType.Tanh
                                if g == 3
                                else mybir.ActivationFunctionType.Sigmoid
                            )
                            nc.scalar.activation(act_t[:], acc[:], func, bias=bias_t[:, g : g + 1])
                            gate_sb.append(act_t)

                        i_t, f_t, o_t, g_t = gate_sb
                        ig = opool.tile([C_HID, 512], f32, tag="ig")
                        nc.vector.tensor_mul(ig[:], i_t[:], g_t[:])
                        fc = opool.tile([C_HID, 512], f32, tag="fc")
                        nc.vector.tensor_mul(fc[:], f_t[:], c_sl[:])
                        cn_t = opool.tile([C_HID, 512], f32, tag="cn")
                        nc.vector.tensor_add(cn_t[:], fc[:], ig[:])
                        nc.sync.dma_start(cn_ap[b][:, h0 * W : (h0 + ROWS_PER_CHUNK) * W], cn_t[:])
                        th_t = opool.tile([C_HID, 512], f32, tag="th")
                        nc.scalar.activation(th_t[:], cn_t[:], mybir.ActivationFunctionType.Tanh)
                        hn_t = opool.tile([C_HID, 512], f32, tag="hn")
                        nc.vector.tensor_mul(hn_t[:], o_t[:], th_t[:])
                        nc.sync.dma_start(hn_ap[b][:, h0 * W : (h0 + ROWS_PER_CHUNK) * W], hn_t[:])

    nc.compile()
    _cache[key] = nc
    return nc


def _prep_inputs(x, h_cur, c_cur, weight, bias, dt_mm):
    """Host-side reshape/shard. Returns in_maps for the 8 cores."""
    if dt_mm == mybir.dt.bfloat16:
        import ml_dtypes

        cast = lambda a: np.asarray(a, dtype=ml_dtypes.bfloat16)
    elif dt_mm == mybir.dt.float16:
        cast = lambda a: np.asarray(a, dtype=np.float16)
    else:
        cast = lambda a: np.ascontiguousarray(a, dtype=np.float32)

    # weight: [4*C_HID, C_IN + C_HID, 3, 3] -> [tap, ci, co]
    wt = np.ascontiguousarray(weight.transpose(2, 3, 1, 0))  # [3,3,ci,co]
    wx = wt[:, :, :C_IN, :]  # [3,3,64,512]
    wh = cast(wt[:, :, C_IN:, :].reshape(9, C_HID, 4 * C_HID))
    # wxp[kh] = stacked [wx(kh,0); wx(kh,1)] for the paired K=128 matmul
    wxp = cast(np.concatenate([wx[:, 0, :, :], wx[:, 1, :, :]], axis=1))  # [3,128,512]
    # wxr = stacked [wx(0,2); wx(1,2)] for the row-paired K=128 matmul
    wxr = cast(np.concatenate([wx[0, 2, :, :], wx[1, 2, :, :]], axis=0))  # [128,512]
    wx3 = cast(np.ascontiguousarray(wx[2, 2, :, :]))  # [64,512]
    biasT = np.ascontiguousarray(bias.reshape(4, C_HID).T, dtype=np.float32)
    c3 = np.ascontiguousarray(c_cur.reshape(B, C_HID, H * W), dtype=np.float32)

    # zero-pad x/h on host so the device load is one contiguous DMA per image.
    # x channels 64-127 hold the same image shifted one column left, so taps
    # (kh,0) and (kh,1) contract in a single K=128 matmul. x2's channels
    # 64-127 are shifted one row up, pairing (0,2) with (1,2).
    xpad = np.zeros((B, 2 * C_IN, HP, WP), dtype=np.float32)
    xpad[:, :C_IN, 1 : H + 1, 1 : W + 1] = x
    xpad[:, C_IN:, :, : WP - 1] = xpad[:, :C_IN, :, 1:]
    x2pad = np.zeros((B, 2 * C_IN, HP, WP), dtype=np.float32)
    x2pad[:, :C_IN] = xpad[:, :C_IN]
    x2pad[:, C_IN:, : HP - 1, :] = xpad[:, :C_IN, 1:, :]
    hpad = np.zeros((B, C_HID, HP, WP), dtype=np.float32)
    hpad[:, :, 1 : H + 1, 1 : W + 1] = h_cur

    in_maps = []
    for i in range(N_CORES):
        s = slice(i * B_LOC, (i + 1) * B_LOC)
        in_maps.append(
            {
                "x": cast(xpad[s]),
                "x2": cast(x2pad[s]),
                "h": cast(hpad[s]),
                "c": c3[s],
                "wxp": wxp,
                "wxr": wxr,
                "wx3": wx3,
                "wh": wh,
                "biasT": biasT,
            }
        )
    return in_maps


def run(x, h_cur, c_cur, weight, bias, dt_mm=mybir.dt.float16, trace=False):
    x = np.asarray(x)
    h_cur = np.asarray(h_cur)
    c_cur = np.asarray(c_cur)
    weight = np.asarray(weight)
    bias = np.asarray(bias)
    nc = _build(dt_mm, trace)
    in_maps = _prep_inputs(x, h_cur, c_cur, weight, bias, dt_mm)
    res = run_bass_kernel_spmd(nc, in_maps, list(range(N_CORES)), trace=trace)
    hn = np.concatenate([res.results[i]["hn"] for i in range(N_CORES)], axis=0)
    cn = np.concatenate([res.results[i]["cn"] for i in range(N_CORES)], axis=0)
    hn = hn.reshape(B, C_HID, H, W).astype(np.float32)
    cn = cn.reshape(B, C_HID, H, W).astype(np.float32)
    return (hn, cn), res


def kernel(x, h_cur, c_cur, weight, bias):
    (hn, cn), _ = run(x, h_cur, c_cur, weight, bias)
    return hn, cn


def _make_timing_fn(nc, in_maps):
    """Non-donating jitted runner with device-resident inputs, for
    throughput timing (slope of wall time vs iteration count)."""
    import jax
    from jax.sharding import NamedSharding

    from concourse import bass2jax, mybir as _mybir

    bass2jax.install_neuronx_cc_hook()
    n_cores = len(in_maps)
    partition_name = nc.partition_id_tensor.name if nc.partition_id_tensor else None
    in_names, out_names, out_avals, zero_outs = [], [], [], []
    for alloc in nc.m.functions[0].allocations:
        if not isinstance(alloc, _mybir.MemoryLocationSet):
            continue
        name = alloc.memorylocations[0].name
        if alloc.kind == "ExternalInput":
            if name != partition_name:
                in_names.append(name)
        elif alloc.kind == "ExternalOutput":
            out_names.append(name)
            shape = tuple(alloc.tensor_shape)
            dtype = _mybir.dt.np(alloc.dtype)
            out_avals.append(jax.core.ShapedArray(shape, dtype))
            zero_outs.append(np.zeros(shape, dtype))
    n_params = len(in_names)
    all_in_names = list(in_names) + list(out_names)
    if partition_name is not None:
        all_in_names.append(partition_name)

    def _body(*args):
        operands = list(args)
        if partition_name is not None:
            operands.append(bass2jax.partition_id_tensor())
        outs = bass2jax._bass_exec_p.bind(
            *operands,
            out_avals=tuple(out_avals),
            in_names=tuple(all_in_names),
            out_names=tuple(out_names),
            lowering_input_output_aliases=(),
            sim_require_finite=True,
            sim_require_nnan=True,
            nc=nc,
        )
        return tuple(outs)

    devices = jax.devices()[:n_cores]
    mesh = bass2jax.Mesh(np.asarray(devices), ("core",))
    in_specs = (bass2jax.PartitionSpec("core"),) * (n_params + len(out_names))
    out_specs = (bass2jax.PartitionSpec("core"),) * len(out_names)
    fn = jax.jit(
        bass2jax.shard_map(
            _body, mesh=mesh, in_specs=in_specs, out_specs=out_specs, check_rep=False
        ),
        keep_unused=True,
    )
    per_core = [[np.asarray(m[name]) for name in in_names] for m in in_maps]
    concat_in = [
        np.concatenate([per_core[c][i] for c in range(n_cores)], axis=0)
        for i in range(n_params)
    ]
    concat_zeros = [
        np.zeros((n_cores * z.shape[0], *z.shape[1:]), z.dtype) for z in zero_outs
    ]
    sh = NamedSharding(mesh, bass2jax.PartitionSpec("core"))
    dev_args = [jax.device_put(a, sh) for a in concat_in + concat_zeros]
    return fn, dev_args


def bench(x, h_cur, c_cur, weight, bias, dt_mm=None, ks=(4, 16)):
    """Returns estimated per-call device exec time in ns (pipelined slope)."""
    import time as _time

    import jax

    if dt_mm is None:
        dt_mm = mybir.dt.float16
    nc = _build(dt_mm)
    in_maps = _prep_inputs(
        np.asarray(x), np.asarray(h_cur), np.asarray(c_cur), np.asarray(weight), np.asarray(bias), dt_mm
    )
    fn, dev_args = _make_timing_fn(nc, in_maps)
    # warmup (compile + first exec)
    for _ in range(2):
        outs = fn(*dev_args)
        jax.block_until_ready(outs)

    def timed(k):
        t0 = _time.perf_counter()
        outs = None
        for _ in range(k):
            outs = fn(*dev_args)
        jax.block_until_ready(outs)
        return _time.perf_counter() - t0

    times = {}
    for k in ks:
        times[k] = min(timed(k) for _ in range(3))
    k_lo, k_hi = min(ks), max(ks)
    slope = (times[k_hi] - times[k_lo]) / (k_hi - k_lo)
    return slope * 1e9, times
